# revision 35
# baseline (speedup 1.0000x reference)
"""CTC loss kernel for Trainium2 (Bass/Tile), 8-core data-parallel.

Per core (8 samples): linear-space CTC forward recurrence, scanned
column-by-column over the extended-label axis (S=201).  The time axis
(T=1000) lives on the free dim, split into 4 chunks of 250 mapped to the
four SBUF partition quadrants (partition = 32*chunk + sample).  Each
column costs 4 chained tensor_tensor_scan instructions (hardware linear
recurrence) + 3 tiny cross-chunk carry ops, plus one fused C-op on odd
(label) columns only: even (blank) columns have skip == 0, so their
scan reads the previous column's tile directly.  DVE ops must start at
32-aligned partitions on TRN2, which is why chunks live at quadrant
bases and finer chunking (or cross-partition carry chains at +-8 rows)
is not expressible.

Numerics: per-frame normalizer nu[t] = (1.2/(2l+1)) * sum_s y_pred[t,ext[s]]
(folded into the w matmul vector host-side) keeps drift to a random walk;
per-(sample,chunk) rescales every 16 columns (never scaling up, rho
exponent clamped, Ln computed with a 2^-32 prescale) keep everything in
f32; the final loss re-adds the log-nu prefix sum (N8) and the
accumulated log scales (lambda).  Validated in f32 vs the reference:
rel err ~2e-7.

Host side does only label-index bookkeeping (one-hot gather matrices,
masks) plus a pure layout transpose of y_pred; all y_pred-dependent math
runs on device.
"""
import os
import sys

sys.path.insert(0, "/opt/trn_rl_repo")

import numpy as np

import concourse.bass as bass
import concourse.bacc as bacc
import concourse.mybir as mybir
import concourse.tile as tile
from concourse.bass_utils import run_bass_kernel_spmd

B, T, C, L = 64, 1000, 128, 100
S = 2 * L + 1            # 201
NB = 8                   # samples per core
NCORE = 8
NCH, TC = 4, 250         # time chunks x chunk length
RS = 16                  # rescale every RS columns
SBLK = 24                # columns per streamed block
KAPPA = 1.2              # normalizer constant (per-sample cK = KAPPA/(2l+1))
LN232 = 22.18070977791825   # 32*ln(2)
LN2 = 0.6931471805599453
EXPMASK = 0x7F800000
RCPBASE = 0x7F000000
I32 = None  # set below
F32 = mybir.dt.float32
INT32 = mybir.dt.int32
AOP = mybir.AluOpType
AFT = mybir.ActivationFunctionType

# big tensor column offsets (partition dim = C = 128)
YP0 = 0                  # ypT: col b*1000 + t
G0 = NB * T              # g:  col G0 + b*201 + s
W0 = G0 + NB * S         # w:  col W0 + b
BIGN = W0 + NB

# aux tensor column offsets (partition dim = 128)
SKP0 = 0                 # skipm [128, S]
EM0 = SKP0 + S           # emask [128, TC+1]
VE0 = EM0 + TC + 1       # veps [128, 2*NB]
I00 = VE0 + 2 * NB       # ind0 [128, 2*NB]
SEL0 = I00 + 2 * NB      # sel [128, NB]
KB0 = SEL0 + NB          # per-sample N8 offset const [rows 0:8, 1]
TM0 = KB0 + 1            # tmask [rows 0:16 = 2b+h, 500]
PS0 = TM0 + 500          # pair-select [rows 0:16, NB]
AUXN = PS0 + NB

_cached = {}


def _build_program():
    from contextlib import ExitStack

    nc = bacc.Bacc(None, target_bir_lowering=False)

    big_d = nc.dram_tensor("big", [C, BIGN], F32, kind="ExternalInput")
    aux_d = nc.dram_tensor("aux", [128, AUXN], F32, kind="ExternalInput")
    loss_d = nc.dram_tensor("loss", [NB, 1], F32, kind="ExternalOutput")
    dbg_d = nc.dram_tensor("dbg", [4, 128], F32, kind="ExternalOutput")
    qhat_d = nc.dram_tensor("qhat", [NB, S, T], F32)   # internal bounce
    scr_d = nc.dram_tensor("scr", [1, 16 * 512], F32)  # nu transpose bounce

    PS_SIG = (128, S - 128)
    HB = 4                    # samples per gather PSUM round

    with tile.TileContext(nc) as tc, ExitStack() as ctx:
        pers = ctx.enter_context(tc.tile_pool(name="pers", bufs=1))
        pbig = ctx.enter_context(tc.tile_pool(name="pbig", bufs=1, space="PSUM"))
        psml = ctx.enter_context(tc.tile_pool(name="psml", bufs=2, space="PSUM"))
        qblk_pool = ctx.enter_context(tc.tile_pool(name="qblk", bufs=2))

        big = pers.tile([C, BIGN], F32, tag="big")
        aux = pers.tile([128, AUXN], F32, tag="aux")
        ones128 = pers.tile([1, 128], F32, tag="ones128")
        nurepS = pers.tile([128, 2, NB, 500], F32, tag="nurepS")
        stag = pers.tile([128, NB, 500], F32, tag="stag")
        junk = pers.tile([128, TC + 1], F32, tag="junk")
        nucat = pers.tile([1, 16 * 512], F32, tag="nucat")
        rncat = nucat  # reused after the row ops (strictly ordered)
        nub = pers.tile([16, 512], F32, tag="nub")
        nubi = pers.tile([16, 512], INT32, tag="nubi")
        rnub = pers.tile([16, 512], F32, tag="rnub")
        efb = pers.tile([16, 512], F32, tag="efb")
        n8col = pers.tile([16, 1], F32, tag="n8col")
        X = []
        for i in range(3):
            xt = pers.tile([128, TC + 1], F32, tag=f"X{i}", name=f"X{i}")
            X.append(xt)
        Cbuf = pers.tile([128, TC], F32, tag="Cbuf")
        sc = pers.tile([128, 8], F32, tag="sc")
        LAM, RHO, LSH, MRE, TMP, R199, R200, RTOT = range(8)
        sci = pers.tile([128, 8], INT32, tag="sci")
        LAMI, LSHI, DI, EI, RA, RB = range(6)
        lamF = pers.tile([128, 1], F32, tag="lamF")
        v8 = pers.tile([NB, 4], F32, tag="v8")
        N8c, LOGRc, T1c, LOSSc = range(4)

        def ypr(b, h):
            return big[:, YP0 + b * T + h * 500:YP0 + b * T + (h + 1) * 500]

        # ---- loads ----
        nc.sync.dma_start(big[:], big_d[:])
        nc.sync.dma_start(aux[:], aux_d[:])
        nc.gpsimd.memset(ones128[:], 1.0)
        nc.gpsimd.memset(junk[:], 0.0)
        nc.gpsimd.memset(nucat[:], 0.0)
        nc.gpsimd.memset(rnub[:], 0.0)

        # ---- nu phase (batched: 16 (b,h) rows on 16 partitions) ----
        for b in range(NB):
            for h in range(2):
                r = 2 * b + h
                pnu = psml.tile([1, 512], F32, tag="psm")
                nc.tensor.matmul(pnu[0:1, 0:500], big[:, W0 + b:W0 + b + 1],
                                 ypr(b, h), start=True, stop=True)
                nc.scalar.copy(nucat[0:1, r * 512:r * 512 + 500],
                               pnu[0:1, 0:500])
        # transpose the 16 nu rows onto 16 partitions via a DRAM bounce.
        # The (r t) split must live on the DRAM-side AP only: an SBUF AP
        # whose partition dim is really free-dim offsets of partition 0
        # reads fine in CoreSim but mis-describes DMA on hardware.
        nc.sync.dma_start(scr_d[0:1, :], nucat[0:1, :])
        nc.sync.dma_start(nub[:, :],
                          scr_d[0:1, :].rearrange("o (r t) -> (o r) t", r=16))
        # exponent bits of nu (power-of-two normalizer)
        nc.vector.tensor_scalar(nubi[:, 0:500], nub.bitcast(INT32)[:, 0:500],
                                EXPMASK, None, AOP.bitwise_and)
        # exact reciprocal: bits = RCPBASE - expbits; mask to t < input_len
        nc.vector.tensor_scalar(rnub.bitcast(INT32)[:, 0:500], nubi[:, 0:500],
                                -1, RCPBASE, AOP.mult, AOP.add)
        nc.vector.tensor_tensor(rnub[:, 0:500], rnub[:, 0:500],
                                aux[0:16, TM0:TM0 + 500], AOP.mult)
        # E field as f32, masked + reduced for the N8 sum
        nc.vector.tensor_scalar(nubi[:, 0:500], nubi[:, 0:500],
                                23, None, AOP.logical_shift_right)
        nc.vector.tensor_copy(efb[:, 0:500], nubi[:, 0:500])
        nc.vector.tensor_tensor(efb[:, 0:500], efb[:, 0:500],
                                aux[0:16, TM0:TM0 + 500], AOP.mult)
        nc.vector.tensor_reduce(n8col[:, 0:1], efb[:, 0:500],
                                mybir.AxisListType.X, AOP.add)
        # back to row-cat layout so broadcast-matmul rhs sits at partition 0
        nc.sync.dma_start(scr_d[0:1, :]
                          .rearrange("o (r t) -> (o r) t", r=16), rnub[:, :])
        nc.sync.dma_start(rncat[0:1, :], scr_d[0:1, :])
        for b in range(NB):
            for h in range(2):
                r = 2 * b + h
                prep = psml.tile([128, 512], F32, tag="prep")
                nc.tensor.matmul(prep[:, 0:500], ones128[:],
                                 rncat[0:1, r * 512:r * 512 + 500],
                                 start=True, stop=True)
                nc.scalar.copy(nurepS[:, h, b, :], prep[:, 0:500])


        # ---- Y prescale ----
        # Y <- (Y + eps) * nurep in place.  The one-hot gather then yields
        # (gathered + eps*valid) * rnu directly (sum_c g[c,s] = valid[s]),
        # so the 32 per-(sig,h,b) DVE fixup ops collapse into 2 passes and
        # the PSUM->SBUF move shifts to the idle Act engine.
        yv = big[:, YP0:YP0 + NB * T].rearrange("p (b h t) -> p b h t",
                                                b=NB, h=2)
        for h in range(2):
            nc.vector.scalar_tensor_tensor(
                yv[:, :, h, :], yv[:, :, h, :], 1e-7,
                nurepS[:, h, :, :], AOP.add, AOP.mult)

        # ---- gather phases ----
        for sig in range(2):
            ps = PS_SIG[sig]
            s0 = 0 if sig == 0 else 128
            for h in range(2):
                for hb in range(2):
                    gat = pbig.tile([128, HB, 512], F32, tag="gat")
                    for bb in range(HB):
                        b = hb * HB + bb
                        nc.tensor.matmul(
                            gat[0:ps, bb, 0:500],
                            big[:, G0 + b * S + s0:G0 + b * S + s0 + ps],
                            ypr(b, h), start=True, stop=True)
                    for bb in range(HB):
                        b = hb * HB + bb
                        nc.scalar.copy(stag[0:ps, b, :],
                                       gat[0:ps, bb, 0:500])
                if h == 0:
                    nc.vector.tensor_tensor(
                        stag[0:ps, :, 0], stag[0:ps, :, 0],
                        aux[0:ps, I00 + NB * sig:I00 + NB * (sig + 1)],
                        AOP.mult)
                nc.sync.dma_start(
                    qhat_d[:, s0:s0 + ps, h * 500:(h + 1) * 500]
                    .rearrange("b s j -> s b j"),
                    stag[0:ps, :, :])
        
        # ---- scan phase ----
        for i in range(3):
            nc.gpsimd.memset(X[i][:], 0.0)
            nc.gpsimd.memset(X[i][0:NB, 0:1], 1.0)
        nc.gpsimd.memset(sc[:], 0.0)
        nc.gpsimd.memset(sci[:], 0)
        nc.gpsimd.memset(sci[:, RB:RB + 1], 0x3F800000)

        sblocks = []
        s = 0
        while s < S:
            n = min(SBLK, S - s)
            if S - (s + n) == 1:
                n += 1
            sblocks.append((s, n))
            s += n

        for (sb, nsb) in sblocks:
            qblk = qblk_pool.tile([128, nsb, TC], F32, tag="qblk")
            for c in range(NCH):
                nc.sync.dma_start(
                    qblk[32 * c:32 * c + NB, :, :],
                    qhat_d[:, sb:sb + nsb, c * TC:(c + 1) * TC])
            for k in range(nsb):
                s = sb + k
                xs = X[s % 3]
                xm1 = X[(s + 2) % 3]
                xm2 = X[(s + 1) % 3]
                if s % 2 == 1:
                    nc.vector.scalar_tensor_tensor(
                        Cbuf[:], xm2[:, 0:TC], aux[:, SKP0 + s:SKP0 + s + 1],
                        xm1[:, 0:TC], AOP.mult, AOP.add)
                    d0 = Cbuf
                else:
                    d0 = xm1
                for c in range(NCH):
                    lo = 32 * c
                    init = 0.0 if c == 0 else xs[lo:lo + NB, 0:1]
                    nc.vector.tensor_tensor_scan(
                        xs[lo:lo + NB, 1:TC + 1],
                        d0[lo:lo + NB, 0:TC],
                        qblk[lo:lo + NB, k, :],
                        init, AOP.add, AOP.mult)
                    if c < NCH - 1:
                        nc.vector.tensor_scalar(
                            xs[lo + 32:lo + 40, 0:1],
                            xs[lo:lo + NB, TC:TC + 1],
                            sci.bitcast(F32)[lo + 32:lo + 40, RB:RB + 1],
                            None, AOP.mult)
                if s in (199, 200):
                    rcol = R199 if s == 199 else R200
                    nc.vector.scalar_tensor_tensor(
                        junk[:, 0:TC + 1], xs[:], 1.0,
                        aux[:, EM0:EM0 + TC + 1],
                        AOP.mult, AOP.mult, accum_out=sc[:, rcol:rcol + 1])
                if (s + 1) % RS == 0 and s < 198:
                    nc.vector.tensor_reduce(
                        sc[:, MRE:MRE + 1], xs[:], mybir.AxisListType.X,
                        AOP.max, apply_absolute_value=True)
                    nc.vector.tensor_scalar_max(
                        sc[:, MRE:MRE + 1], sc[:, MRE:MRE + 1], 1.0)
                    # exponent-bit games: exact power-of-two rescale
                    nc.vector.tensor_scalar(
                        sci[:, RA:RA + 1], sc[:, MRE:MRE + 1].bitcast(INT32),
                        EXPMASK, None, AOP.bitwise_and)
                    nc.vector.tensor_scalar(
                        sci[:, RB:RB + 1], sci[:, RA:RA + 1],
                        -1, RCPBASE, AOP.mult, AOP.add)
                    rcpf = sci.bitcast(F32)[:, RB:RB + 1]
                    nc.vector.tensor_scalar_mul(xs[:], xs[:], rcpf)
                    nc.vector.tensor_scalar_mul(xm1[:], xm1[:], rcpf)
                    nc.gpsimd.memset(xs[0:NB, 0:1], 1.0)
                    nc.gpsimd.memset(xm1[0:NB, 0:1], 1.0)
                    nc.vector.tensor_scalar(
                        sci[:, EI:EI + 1], sci[:, RA:RA + 1],
                        23, None, AOP.logical_shift_right)
                    nc.vector.tensor_scalar(
                        sci[:, EI:EI + 1], sci[:, EI:EI + 1],
                        127, None, AOP.subtract)
                    nc.vector.tensor_tensor(sci[:, LAMI:LAMI + 1],
                                            sci[:, LAMI:LAMI + 1],
                                            sci[:, EI:EI + 1], AOP.add)
                    nc.vector.tensor_copy(sci[32:64, LSHI:LSHI + 1],
                                          sci[0:32, LAMI:LAMI + 1])
                    nc.vector.tensor_copy(sci[64:96, LSHI:LSHI + 1],
                                          sci[32:64, LAMI:LAMI + 1])
                    nc.vector.tensor_copy(sci[96:128, LSHI:LSHI + 1],
                                          sci[64:96, LAMI:LAMI + 1])
                    nc.vector.tensor_tensor(sci[:, DI:DI + 1],
                                            sci[:, LSHI:LSHI + 1],
                                            sci[:, LAMI:LAMI + 1],
                                            AOP.subtract)
                    nc.vector.tensor_scalar(sci[:, DI:DI + 1],
                                            sci[:, DI:DI + 1],
                                            126, -126, AOP.min, AOP.max)
                    nc.vector.tensor_scalar(sci[:, RB:RB + 1],
                                            sci[:, DI:DI + 1],
                                            127, None, AOP.add)
                    nc.vector.tensor_scalar(sci[:, RB:RB + 1],
                                            sci[:, RB:RB + 1],
                                            23, None, AOP.logical_shift_left)

        # ---- finalize ----
        nc.vector.tensor_tensor(sc[:, RTOT:RTOT + 1], sc[:, R199:R199 + 1],
                                sc[:, R200:R200 + 1], AOP.add)
        pr8 = psml.tile([NB, 512], F32, tag="psm")
        nc.tensor.matmul(pr8[:, 0:1], aux[:, SEL0:SEL0 + NB],
                         sc[:, RTOT:RTOT + 1], start=True, stop=True)
        nc.vector.tensor_copy(lamF[:], sci[:, LAMI:LAMI + 1])
        nc.vector.tensor_scalar_mul(lamF[:], lamF[:], LN2)
        plam8 = psml.tile([NB, 512], F32, tag="prep")
        nc.tensor.matmul(plam8[:, 0:1], aux[:, SEL0:SEL0 + NB],
                         lamF[:], start=True, stop=True)
        pn8 = psml.tile([NB, 512], F32, tag="psm")
        nc.tensor.matmul(pn8[:, 0:1], aux[0:16, PS0:PS0 + NB],
                         n8col[:, 0:1], start=True, stop=True)
        nc.vector.scalar_tensor_tensor(
            v8[:, N8c:N8c + 1], pn8[:, 0:1], LN2,
            aux[0:NB, KB0:KB0 + 1], AOP.mult, AOP.add)
        # split r = m * 2^(E-127), m in [1,2): exact exponent, Ln on mantissa
        ri8 = pers.tile([NB, 2], INT32, tag="ri8")
        rf8 = pers.tile([NB, 2], F32, tag="rf8")
        nc.vector.tensor_scalar(ri8[:, 0:1], pr8[:, 0:1].bitcast(INT32),
                                23, None, AOP.logical_shift_right)
        nc.vector.tensor_copy(rf8[:, 0:1], ri8[:, 0:1])
        nc.vector.tensor_scalar(ri8[:, 1:2], pr8[:, 0:1].bitcast(INT32),
                                0x007FFFFF, 0x3F800000,
                                AOP.bitwise_and, AOP.bitwise_or)
        nc.scalar.activation(v8[:, LOGRc:LOGRc + 1],
                             ri8.bitcast(F32)[:, 1:2], AFT.Ln)
        nc.vector.tensor_scalar(rf8[:, 0:1], rf8[:, 0:1],
                                127.0, LN2, AOP.subtract, AOP.mult)
        nc.vector.tensor_tensor(v8[:, LOGRc:LOGRc + 1],
                                v8[:, LOGRc:LOGRc + 1],
                                rf8[:, 0:1], AOP.add)
        nc.vector.tensor_tensor(v8[:, T1c:T1c + 1], v8[:, LOGRc:LOGRc + 1],
                                v8[:, N8c:N8c + 1], AOP.add)
        nc.vector.scalar_tensor_tensor(
            v8[:, LOSSc:LOSSc + 1], v8[:, T1c:T1c + 1], -1.0, plam8[:, 0:1],
            AOP.mult, AOP.subtract)
        nc.sync.dma_start(loss_d[:], v8[:, LOSSc:LOSSc + 1])
        nc.sync.dma_start(dbg_d[0:1, :].rearrange("o p -> p o"), sc[:, RTOT:RTOT + 1])
        nc.sync.dma_start(dbg_d[1:2, :].rearrange("o p -> p o"), lamF[:])
        nc.sync.dma_start(dbg_d[2, 0:4 * NB].rearrange("(p o) -> p o", o=4),
                          v8[:, :])
        nc.sync.dma_start(dbg_d[3:4, 0:16].rearrange("o p -> p o"),
                          n8col[:, 0:1])

    nc.finalize()
    return nc


def _host_prep(y_true, y_pred, input_lengths, label_lengths):
    in_maps = []
    for core in range(NCORE):
        bsl = slice(core * NB, (core + 1) * NB)
        yt = y_true[bsl]
        ilen = input_lengths[bsl].astype(np.int64)
        llen = label_lengths[bsl].astype(np.int64)

        big = np.zeros((C, BIGN), np.float32)
        big[:, YP0:YP0 + NB * T] = (
            y_pred[bsl].transpose(2, 0, 1).reshape(C, NB * T))
        aux = np.zeros((128, AUXN), np.float32)

        for b in range(NB):
            l = int(llen[b]); o = 200 - 2 * l
            ext = np.full(S, -1, np.int32)
            for k in range(2 * l + 1):
                ext[o + k] = C - 1 if k % 2 == 0 else yt[b, (k - 1) // 2]
            gb = np.zeros((C, S), np.float32)
            for s in range(S):
                if ext[s] >= 0:
                    gb[ext[s], s] = 1.0
                k = s - o
                if k >= 2 and k % 2 == 1 and ext[s] != ext[s - 2]:
                    for c in range(NCH):
                        aux[32 * c + b, SKP0 + s] = 1.0
            big[:, G0 + b * S:G0 + (b + 1) * S] = gb
            big[:, W0 + b] = gb.sum(axis=1) * np.float32(KAPPA * np.sqrt(2.0) / (2 * l + 1))
            for sig in range(2):
                s0, ps = (0, 128) if sig == 0 else (128, S - 128)
                for sp in range(ps):
                    if ext[s0 + sp] >= 0:
                        aux[sp, VE0 + NB * sig + b] = 1e-7
                for tgt in (o, o + 1):
                    if s0 <= tgt < s0 + ps:
                        aux[tgt - s0, I00 + NB * sig + b] = 1.0
            tstar = int(ilen[b]) - 1
            cstar = tstar // TC
            jstar = tstar - cstar * TC + 1
            aux[32 * cstar + b, EM0 + jstar] = 1.0
            aux[32 * cstar + b, SEL0 + b] = 1.0
            aux[b, KB0] = -np.log(2.0) * 127.0 * (tstar + 1)
            for h in range(2):
                r = 2 * b + h
                n_on = min(max(tstar + 1 - 500 * h, 0), 500)
                aux[r, TM0:TM0 + n_on] = 1.0
                aux[r, PS0 + b] = 1.0

        in_maps.append({"big": big, "aux": aux})
    return in_maps


def kernel(y_true, y_pred, input_lengths, label_lengths):
    y_true = np.asarray(y_true)
    y_pred = np.asarray(y_pred, dtype=np.float32)
    input_lengths = np.asarray(input_lengths)
    label_lengths = np.asarray(label_lengths)

    if "nc" not in _cached:
        _cached["nc"] = _build_program()
    nc = _cached["nc"]

    in_maps = _host_prep(y_true, y_pred, input_lengths, label_lengths)
    res = run_bass_kernel_spmd(nc, in_maps, core_ids=list(range(NCORE)))
    out = np.concatenate([res.results[i]["loss"] for i in range(NCORE)], axis=0)
    return out.astype(np.float32)



# revision 37
# speedup vs baseline: 2.9030x; 2.9030x over previous
"""CTC loss kernel for Trainium2 (Bass/Tile), 8-core data-parallel.

Per core (8 samples): linear-space CTC forward recurrence, scanned
column-by-column over the extended-label axis (S=201).  The time axis
(T=1000) lives on the free dim, split into 4 chunks of 250 mapped to the
four SBUF partition quadrants (partition = 32*chunk + sample).  Each
column costs 4 chained tensor_tensor_scan instructions (hardware linear
recurrence) + 3 tiny cross-chunk carry ops, plus one fused C-op on odd
(label) columns only: even (blank) columns have skip == 0, so their
scan reads the previous column's tile directly.  DVE ops must start at
32-aligned partitions on TRN2, which is why chunks live at quadrant
bases and finer chunking (or cross-partition carry chains at +-8 rows)
is not expressible.

Numerics: per-frame normalizer nu[t] = (1.2/(2l+1)) * sum_s y_pred[t,ext[s]]
(folded into the w matmul vector host-side) keeps drift to a random walk;
per-(sample,chunk) rescales every 16 columns (never scaling up, rho
exponent clamped, Ln computed with a 2^-32 prescale) keep everything in
f32; the final loss re-adds the log-nu prefix sum (N8) and the
accumulated log scales (lambda).  Validated in f32 vs the reference:
rel err ~2e-7.

Host side does only label-index bookkeeping (one-hot gather matrices,
masks) plus a pure layout transpose of y_pred; all y_pred-dependent math
runs on device.
"""
import os
import sys

sys.path.insert(0, "/opt/trn_rl_repo")

import numpy as np

import concourse.bass as bass
import concourse.bacc as bacc
import concourse.mybir as mybir
import concourse.tile as tile
from concourse.bass_utils import run_bass_kernel_spmd

B, T, C, L = 64, 1000, 128, 100
S = 2 * L + 1            # 201
NB = 8                   # samples per core
NCORE = 8
NCH, TC = 2, 500         # time chunks x chunk length (quadrants 0 and 1)
RS = 16                  # rescale every RS columns
SBLK = 12                # columns per streamed block (SBUF-limited)
KAPPA = 1.2              # normalizer constant (per-sample cK = KAPPA/(2l+1))
LN232 = 22.18070977791825   # 32*ln(2)
LN2 = 0.6931471805599453
EXPMASK = 0x7F800000
RCPBASE = 0x7F000000
I32 = None  # set below
F32 = mybir.dt.float32
INT32 = mybir.dt.int32
AOP = mybir.AluOpType
AFT = mybir.ActivationFunctionType

# big tensor column offsets (partition dim = C = 128)
YP0 = 0                  # ypT: col b*1000 + t
G0 = NB * T              # g:  col G0 + b*201 + s
W0 = G0 + NB * S         # w:  col W0 + b
BIGN = W0 + NB

# aux tensor column offsets (partition dim = 128)
SKP0 = 0                 # skipm [128, S]
EM0 = SKP0 + S           # emask [128, TC+1]
VE0 = EM0 + TC + 1       # veps [128, 2*NB]
I00 = VE0 + 2 * NB       # ind0 [128, 2*NB]
SEL0 = I00 + 2 * NB      # sel [128, NB]
KB0 = SEL0 + NB          # per-sample N8 offset const [rows 0:8, 1]
TM0 = KB0 + 1            # tmask [rows 0:16 = 2b+h, 500]
PS0 = TM0 + 500          # pair-select [rows 0:16, NB]
AUXN = PS0 + NB

_cached = {}


def _build_program():
    from contextlib import ExitStack

    nc = bacc.Bacc(None, target_bir_lowering=False)

    big_d = nc.dram_tensor("big", [C, BIGN], F32, kind="ExternalInput")
    aux_d = nc.dram_tensor("aux", [128, AUXN], F32, kind="ExternalInput")
    loss_d = nc.dram_tensor("loss", [NB, 1], F32, kind="ExternalOutput")
    dbg_d = nc.dram_tensor("dbg", [4, 128], F32, kind="ExternalOutput")
    qhat_d = nc.dram_tensor("qhat", [NB, S, T], F32)   # internal bounce
    scr_d = nc.dram_tensor("scr", [1, 16 * 512], F32)  # nu transpose bounce

    PS_SIG = (128, S - 128)
    HB = 4                    # samples per gather PSUM round

    with tile.TileContext(nc) as tc, ExitStack() as ctx:
        pers = ctx.enter_context(tc.tile_pool(name="pers", bufs=1))
        pbig = ctx.enter_context(tc.tile_pool(name="pbig", bufs=1, space="PSUM"))
        psml = ctx.enter_context(tc.tile_pool(name="psml", bufs=2, space="PSUM"))
        qblk_pool = ctx.enter_context(tc.tile_pool(name="qblk", bufs=2))

        big = pers.tile([C, BIGN], F32, tag="big")
        aux = pers.tile([128, AUXN], F32, tag="aux")
        ones128 = pers.tile([1, 128], F32, tag="ones128")
        nurepS = pers.tile([128, 2, NB, 500], F32, tag="nurepS")
        stag = pers.tile([128, NB, 500], F32, tag="stag")
        junk = pers.tile([128, TC + 1], F32, tag="junk")
        nucat = pers.tile([1, 16 * 512], F32, tag="nucat")
        rncat = nucat  # reused after the row ops (strictly ordered)
        nub = pers.tile([16, 512], F32, tag="nub")
        nubi = pers.tile([16, 512], INT32, tag="nubi")
        rnub = pers.tile([16, 512], F32, tag="rnub")
        efb = pers.tile([16, 512], F32, tag="efb")
        n8col = pers.tile([16, 1], F32, tag="n8col")
        X = []
        for i in range(3):
            xt = pers.tile([128, TC + 1], F32, tag=f"X{i}", name=f"X{i}")
            X.append(xt)
        Cbuf = pers.tile([128, TC], F32, tag="Cbuf")
        sc = pers.tile([128, 8], F32, tag="sc")
        LAM, RHO, LSH, MRE, TMP, R199, R200, RTOT = range(8)
        sci = pers.tile([128, 8], INT32, tag="sci")
        LAMI, LSHI, DI, EI, RA, RB = range(6)
        lamF = pers.tile([128, 1], F32, tag="lamF")
        v8 = pers.tile([NB, 4], F32, tag="v8")
        N8c, LOGRc, T1c, LOSSc = range(4)

        def ypr(b, h):
            return big[:, YP0 + b * T + h * 500:YP0 + b * T + (h + 1) * 500]

        # ---- loads ----
        nc.sync.dma_start(big[:], big_d[:])
        nc.sync.dma_start(aux[:], aux_d[:])
        nc.gpsimd.memset(ones128[:], 1.0)
        nc.gpsimd.memset(junk[:], 0.0)
        nc.gpsimd.memset(nucat[:], 0.0)
        nc.gpsimd.memset(rnub[:], 0.0)

        # ---- nu phase (batched: 16 (b,h) rows on 16 partitions) ----
        for b in range(NB):
            for h in range(2):
                r = 2 * b + h
                pnu = psml.tile([1, 512], F32, tag="psm")
                nc.tensor.matmul(pnu[0:1, 0:500], big[:, W0 + b:W0 + b + 1],
                                 ypr(b, h), start=True, stop=True)
                nc.scalar.copy(nucat[0:1, r * 512:r * 512 + 500],
                               pnu[0:1, 0:500])
        # transpose the 16 nu rows onto 16 partitions via a DRAM bounce.
        # The (r t) split must live on the DRAM-side AP only: an SBUF AP
        # whose partition dim is really free-dim offsets of partition 0
        # reads fine in CoreSim but mis-describes DMA on hardware.
        nc.sync.dma_start(scr_d[0:1, :], nucat[0:1, :])
        nc.sync.dma_start(nub[:, :],
                          scr_d[0:1, :].rearrange("o (r t) -> (o r) t", r=16))
        # exponent bits of nu (power-of-two normalizer)
        nc.vector.tensor_scalar(nubi[:, 0:500], nub.bitcast(INT32)[:, 0:500],
                                EXPMASK, None, AOP.bitwise_and)
        # exact reciprocal: bits = RCPBASE - expbits; mask to t < input_len
        nc.vector.tensor_scalar(rnub.bitcast(INT32)[:, 0:500], nubi[:, 0:500],
                                -1, RCPBASE, AOP.mult, AOP.add)
        nc.vector.tensor_tensor(rnub[:, 0:500], rnub[:, 0:500],
                                aux[0:16, TM0:TM0 + 500], AOP.mult)
        # E field as f32, masked + reduced for the N8 sum
        nc.vector.tensor_scalar(nubi[:, 0:500], nubi[:, 0:500],
                                23, None, AOP.logical_shift_right)
        nc.vector.tensor_copy(efb[:, 0:500], nubi[:, 0:500])
        nc.vector.tensor_tensor(efb[:, 0:500], efb[:, 0:500],
                                aux[0:16, TM0:TM0 + 500], AOP.mult)
        nc.vector.tensor_reduce(n8col[:, 0:1], efb[:, 0:500],
                                mybir.AxisListType.X, AOP.add)
        # back to row-cat layout so broadcast-matmul rhs sits at partition 0
        nc.sync.dma_start(scr_d[0:1, :]
                          .rearrange("o (r t) -> (o r) t", r=16), rnub[:, :])
        nc.sync.dma_start(rncat[0:1, :], scr_d[0:1, :])
        for b in range(NB):
            for h in range(2):
                r = 2 * b + h
                prep = psml.tile([128, 512], F32, tag="prep")
                nc.tensor.matmul(prep[:, 0:500], ones128[:],
                                 rncat[0:1, r * 512:r * 512 + 500],
                                 start=True, stop=True)
                nc.scalar.copy(nurepS[:, h, b, :], prep[:, 0:500])


        # ---- Y prescale ----
        # Y <- (Y + eps) * nurep in place.  The one-hot gather then yields
        # (gathered + eps*valid) * rnu directly (sum_c g[c,s] = valid[s]),
        # so the 32 per-(sig,h,b) DVE fixup ops collapse into 2 passes and
        # the PSUM->SBUF move shifts to the idle Act engine.
        yv = big[:, YP0:YP0 + NB * T].rearrange("p (b h t) -> p b h t",
                                                b=NB, h=2)
        for h in range(2):
            nc.vector.scalar_tensor_tensor(
                yv[:, :, h, :], yv[:, :, h, :], 1e-7,
                nurepS[:, h, :, :], AOP.add, AOP.mult)

        # ---- gather phases ----
        for sig in range(2):
            ps = PS_SIG[sig]
            s0 = 0 if sig == 0 else 128
            for h in range(2):
                for hb in range(2):
                    gat = pbig.tile([128, HB, 512], F32, tag="gat")
                    for bb in range(HB):
                        b = hb * HB + bb
                        nc.tensor.matmul(
                            gat[0:ps, bb, 0:500],
                            big[:, G0 + b * S + s0:G0 + b * S + s0 + ps],
                            ypr(b, h), start=True, stop=True)
                    for bb in range(HB):
                        b = hb * HB + bb
                        nc.scalar.copy(stag[0:ps, b, :],
                                       gat[0:ps, bb, 0:500])
                if h == 0:
                    nc.vector.tensor_tensor(
                        stag[0:ps, :, 0], stag[0:ps, :, 0],
                        aux[0:ps, I00 + NB * sig:I00 + NB * (sig + 1)],
                        AOP.mult)
                nc.sync.dma_start(
                    qhat_d[:, s0:s0 + ps, h * 500:(h + 1) * 500]
                    .rearrange("b s j -> s b j"),
                    stag[0:ps, :, :])
        
        # ---- scan phase ----
        for i in range(3):
            nc.gpsimd.memset(X[i][:], 0.0)
            nc.gpsimd.memset(X[i][0:NB, 0:1], 1.0)
        nc.gpsimd.memset(sc[:], 0.0)
        nc.gpsimd.memset(sci[:], 0)
        nc.gpsimd.memset(sci[:, RB:RB + 1], 0x3F800000)

        sblocks = []
        s = 0
        while s < S:
            n = min(SBLK, S - s)
            if S - (s + n) == 1:
                n += 1
            sblocks.append((s, n))
            s += n

        for (sb, nsb) in sblocks:
            qblk = qblk_pool.tile([128, nsb, TC], F32, tag="qblk")
            for c in range(NCH):
                nc.sync.dma_start(
                    qblk[32 * c:32 * c + NB, :, :],
                    qhat_d[:, sb:sb + nsb, c * TC:(c + 1) * TC])
            for k in range(nsb):
                s = sb + k
                xs = X[s % 3]
                xm1 = X[(s + 2) % 3]
                xm2 = X[(s + 1) % 3]
                if s % 2 == 1:
                    nc.vector.scalar_tensor_tensor(
                        Cbuf[:], xm2[:, 0:TC], aux[:, SKP0 + s:SKP0 + s + 1],
                        xm1[:, 0:TC], AOP.mult, AOP.add)
                    d0 = Cbuf
                else:
                    d0 = xm1
                for c in range(NCH):
                    lo = 32 * c
                    init = 0.0 if c == 0 else xs[lo:lo + NB, 0:1]
                    nc.vector.tensor_tensor_scan(
                        xs[lo:lo + NB, 1:TC + 1],
                        d0[lo:lo + NB, 0:TC],
                        qblk[lo:lo + NB, k, :],
                        init, AOP.add, AOP.mult)
                    if c < NCH - 1:
                        nc.vector.tensor_scalar(
                            xs[lo + 32:lo + 40, 0:1],
                            xs[lo:lo + NB, TC:TC + 1],
                            sci.bitcast(F32)[lo + 32:lo + 40, RB:RB + 1],
                            None, AOP.mult)
                if s in (199, 200):
                    rcol = R199 if s == 199 else R200
                    nc.vector.scalar_tensor_tensor(
                        junk[:, 0:TC + 1], xs[:], 1.0,
                        aux[:, EM0:EM0 + TC + 1],
                        AOP.mult, AOP.mult, accum_out=sc[:, rcol:rcol + 1])
                if (s + 1) % RS == 0 and s < 198:
                    nc.vector.tensor_reduce(
                        sc[:, MRE:MRE + 1], xs[:], mybir.AxisListType.X,
                        AOP.max, apply_absolute_value=True)
                    nc.vector.tensor_scalar_max(
                        sc[:, MRE:MRE + 1], sc[:, MRE:MRE + 1], 1.0)
                    # exponent-bit games: exact power-of-two rescale
                    nc.vector.tensor_scalar(
                        sci[:, RA:RA + 1], sc[:, MRE:MRE + 1].bitcast(INT32),
                        EXPMASK, None, AOP.bitwise_and)
                    nc.vector.tensor_scalar(
                        sci[:, RB:RB + 1], sci[:, RA:RA + 1],
                        -1, RCPBASE, AOP.mult, AOP.add)
                    rcpf = sci.bitcast(F32)[:, RB:RB + 1]
                    nc.vector.tensor_scalar_mul(xs[:], xs[:], rcpf)
                    nc.vector.tensor_scalar_mul(xm1[:], xm1[:], rcpf)
                    nc.gpsimd.memset(xs[0:NB, 0:1], 1.0)
                    nc.gpsimd.memset(xm1[0:NB, 0:1], 1.0)
                    nc.vector.tensor_scalar(
                        sci[:, EI:EI + 1], sci[:, RA:RA + 1],
                        23, None, AOP.logical_shift_right)
                    nc.vector.tensor_scalar(
                        sci[:, EI:EI + 1], sci[:, EI:EI + 1],
                        127, None, AOP.subtract)
                    nc.vector.tensor_tensor(sci[:, LAMI:LAMI + 1],
                                            sci[:, LAMI:LAMI + 1],
                                            sci[:, EI:EI + 1], AOP.add)
                    nc.vector.tensor_copy(sci[32:64, LSHI:LSHI + 1],
                                          sci[0:32, LAMI:LAMI + 1])
                    nc.vector.tensor_copy(sci[64:96, LSHI:LSHI + 1],
                                          sci[32:64, LAMI:LAMI + 1])
                    nc.vector.tensor_copy(sci[96:128, LSHI:LSHI + 1],
                                          sci[64:96, LAMI:LAMI + 1])
                    nc.vector.tensor_tensor(sci[:, DI:DI + 1],
                                            sci[:, LSHI:LSHI + 1],
                                            sci[:, LAMI:LAMI + 1],
                                            AOP.subtract)
                    nc.vector.tensor_scalar(sci[:, DI:DI + 1],
                                            sci[:, DI:DI + 1],
                                            126, -126, AOP.min, AOP.max)
                    nc.vector.tensor_scalar(sci[:, RB:RB + 1],
                                            sci[:, DI:DI + 1],
                                            127, None, AOP.add)
                    nc.vector.tensor_scalar(sci[:, RB:RB + 1],
                                            sci[:, RB:RB + 1],
                                            23, None, AOP.logical_shift_left)

        # ---- finalize ----
        nc.vector.tensor_tensor(sc[:, RTOT:RTOT + 1], sc[:, R199:R199 + 1],
                                sc[:, R200:R200 + 1], AOP.add)
        pr8 = psml.tile([NB, 512], F32, tag="psm")
        nc.tensor.matmul(pr8[:, 0:1], aux[:, SEL0:SEL0 + NB],
                         sc[:, RTOT:RTOT + 1], start=True, stop=True)
        nc.vector.tensor_copy(lamF[:], sci[:, LAMI:LAMI + 1])
        nc.vector.tensor_scalar_mul(lamF[:], lamF[:], LN2)
        plam8 = psml.tile([NB, 512], F32, tag="prep")
        nc.tensor.matmul(plam8[:, 0:1], aux[:, SEL0:SEL0 + NB],
                         lamF[:], start=True, stop=True)
        pn8 = psml.tile([NB, 512], F32, tag="psm")
        nc.tensor.matmul(pn8[:, 0:1], aux[0:16, PS0:PS0 + NB],
                         n8col[:, 0:1], start=True, stop=True)
        nc.vector.scalar_tensor_tensor(
            v8[:, N8c:N8c + 1], pn8[:, 0:1], LN2,
            aux[0:NB, KB0:KB0 + 1], AOP.mult, AOP.add)
        # split r = m * 2^(E-127), m in [1,2): exact exponent, Ln on mantissa
        ri8 = pers.tile([NB, 2], INT32, tag="ri8")
        rf8 = pers.tile([NB, 2], F32, tag="rf8")
        nc.vector.tensor_scalar(ri8[:, 0:1], pr8[:, 0:1].bitcast(INT32),
                                23, None, AOP.logical_shift_right)
        nc.vector.tensor_copy(rf8[:, 0:1], ri8[:, 0:1])
        nc.vector.tensor_scalar(ri8[:, 1:2], pr8[:, 0:1].bitcast(INT32),
                                0x007FFFFF, 0x3F800000,
                                AOP.bitwise_and, AOP.bitwise_or)
        nc.scalar.activation(v8[:, LOGRc:LOGRc + 1],
                             ri8.bitcast(F32)[:, 1:2], AFT.Ln)
        nc.vector.tensor_scalar(rf8[:, 0:1], rf8[:, 0:1],
                                127.0, LN2, AOP.subtract, AOP.mult)
        nc.vector.tensor_tensor(v8[:, LOGRc:LOGRc + 1],
                                v8[:, LOGRc:LOGRc + 1],
                                rf8[:, 0:1], AOP.add)
        nc.vector.tensor_tensor(v8[:, T1c:T1c + 1], v8[:, LOGRc:LOGRc + 1],
                                v8[:, N8c:N8c + 1], AOP.add)
        nc.vector.scalar_tensor_tensor(
            v8[:, LOSSc:LOSSc + 1], v8[:, T1c:T1c + 1], -1.0, plam8[:, 0:1],
            AOP.mult, AOP.subtract)
        nc.sync.dma_start(loss_d[:], v8[:, LOSSc:LOSSc + 1])
        nc.sync.dma_start(dbg_d[0:1, :].rearrange("o p -> p o"), sc[:, RTOT:RTOT + 1])
        nc.sync.dma_start(dbg_d[1:2, :].rearrange("o p -> p o"), lamF[:])
        nc.sync.dma_start(dbg_d[2, 0:4 * NB].rearrange("(p o) -> p o", o=4),
                          v8[:, :])
        nc.sync.dma_start(dbg_d[3:4, 0:16].rearrange("o p -> p o"),
                          n8col[:, 0:1])

    nc.finalize()
    return nc


def _host_prep(y_true, y_pred, input_lengths, label_lengths):
    in_maps = []
    for core in range(NCORE):
        bsl = slice(core * NB, (core + 1) * NB)
        yt = y_true[bsl]
        ilen = input_lengths[bsl].astype(np.int64)
        llen = label_lengths[bsl].astype(np.int64)

        big = np.zeros((C, BIGN), np.float32)
        big[:, YP0:YP0 + NB * T] = (
            y_pred[bsl].transpose(2, 0, 1).reshape(C, NB * T))
        aux = np.zeros((128, AUXN), np.float32)

        for b in range(NB):
            l = int(llen[b]); o = 200 - 2 * l
            ext = np.full(S, -1, np.int32)
            for k in range(2 * l + 1):
                ext[o + k] = C - 1 if k % 2 == 0 else yt[b, (k - 1) // 2]
            gb = np.zeros((C, S), np.float32)
            for s in range(S):
                if ext[s] >= 0:
                    gb[ext[s], s] = 1.0
                k = s - o
                if k >= 2 and k % 2 == 1 and ext[s] != ext[s - 2]:
                    for c in range(NCH):
                        aux[32 * c + b, SKP0 + s] = 1.0
            big[:, G0 + b * S:G0 + (b + 1) * S] = gb
            big[:, W0 + b] = gb.sum(axis=1) * np.float32(KAPPA * np.sqrt(2.0) / (2 * l + 1))
            for sig in range(2):
                s0, ps = (0, 128) if sig == 0 else (128, S - 128)
                for sp in range(ps):
                    if ext[s0 + sp] >= 0:
                        aux[sp, VE0 + NB * sig + b] = 1e-7
                for tgt in (o, o + 1):
                    if s0 <= tgt < s0 + ps:
                        aux[tgt - s0, I00 + NB * sig + b] = 1.0
            tstar = int(ilen[b]) - 1
            cstar = tstar // TC
            jstar = tstar - cstar * TC + 1
            aux[32 * cstar + b, EM0 + jstar] = 1.0
            aux[32 * cstar + b, SEL0 + b] = 1.0
            aux[b, KB0] = -np.log(2.0) * 127.0 * (tstar + 1)
            for h in range(2):
                r = 2 * b + h
                n_on = min(max(tstar + 1 - 500 * h, 0), 500)
                aux[r, TM0:TM0 + n_on] = 1.0
                aux[r, PS0 + b] = 1.0

        in_maps.append({"big": big, "aux": aux})
    return in_maps


def kernel(y_true, y_pred, input_lengths, label_lengths):
    y_true = np.asarray(y_true)
    y_pred = np.asarray(y_pred, dtype=np.float32)
    input_lengths = np.asarray(input_lengths)
    label_lengths = np.asarray(label_lengths)

    if "nc" not in _cached:
        _cached["nc"] = _build_program()
    nc = _cached["nc"]

    in_maps = _host_prep(y_true, y_pred, input_lengths, label_lengths)
    res = run_bass_kernel_spmd(nc, in_maps, core_ids=list(range(NCORE)))
    out = np.concatenate([res.results[i]["loss"] for i in range(NCORE)], axis=0)
    return out.astype(np.float32)

